# revision 7
# baseline (speedup 1.0000x reference)
"""Trainium2 Bass kernel for group-quantized linear layer (GCLIQuantizedLinear).

Computes out[b,s,k] = sum_n x[b,s,n] * W_deq[k,n] + bias[k] where
W_deq = ((W_q - zeros) * scales) * mu2[:,None] * mu1[None,:].

Sharding: data-parallel over the 8192 tokens (M) across 8 cores; every core
holds the full weight matrix. Per core:
  - x shard arrives transposed [N=4096, M=1024] fp32, scaled by mu1 and cast
    to bf16 on device (per-partition tensor_scalar).
  - W_q arrives as W^T bf16 (values 0..15, lossless), host-swizzled so each
    128-column k-chunk is one contiguous 1 MiB DMA in the exact SBUF layout
    [128 n-part, 32 n-tiles, 128 k].
  - Dequant on DVE: W2 = Q * s' + b' with s' = scales*mu2, b' = -zeros*scales*mu2,
    materialized as [128, ...] tiles via partition-broadcast DMAs (stride-0 src).
  - TensorE: out^T[k-chunk, m] accumulated over 32 n-tiles in PSUM,
    bias added during PSUM->SBUF evacuation (per-partition tensor_scalar_add).
Host reassembles out^T columns -> [8192, 4096] -> [4,2048,4096].
"""

import sys

if "/opt/trn_rl_repo" not in sys.path:
    sys.path.insert(0, "/opt/trn_rl_repo")

import numpy as np
import ml_dtypes

import concourse.bass as bass
import concourse.tile as tile
from concourse import mybir, bacc
from concourse.bass_utils import run_bass_kernel_spmd

BF16 = ml_dtypes.bfloat16

P = 128          # partitions
N = 4096         # input features (contraction)
K = 4096         # output features
M_TOT = 8192     # tokens (4*2048)
NCORES = 8
M = M_TOT // NCORES          # 1024 tokens per core
NT = N // P                  # 32 n-tiles (contraction tiles)
NCH = K // P                 # 32 k-chunks of width 128
L = NT * P                   # 4096 free elems in a w-stripe
GS = 64                      # quant group size
FREE = 512                   # matmul moving free dim (one PSUM bank)

_NC_CACHE = None


def _build_program():
    nc = bacc.Bacc("TRN2", target_bir_lowering=False, debug=False)

    xT_d = nc.dram_tensor("xT", [N, M], mybir.dt.float32, kind="ExternalInput")
    wTs_d = nc.dram_tensor("wTs", [NCH, P, L], mybir.dt.bfloat16, kind="ExternalInput")
    zsbc_d = nc.dram_tensor("zsbc", [NCH, 2, P, L], mybir.dt.bfloat16, kind="ExternalInput")
    mu1_d = nc.dram_tensor("mu1c", [P, NT], mybir.dt.float32, kind="ExternalInput")
    bias_d = nc.dram_tensor("biasc", [P, NCH], mybir.dt.float32, kind="ExternalInput")
    outT_d = nc.dram_tensor("outT", [K, M], mybir.dt.float32, kind="ExternalOutput")

    with tile.TileContext(nc) as tc:
        with (
            tc.tile_pool(name="const", bufs=1) as constp,
            tc.tile_pool(name="xstage", bufs=3) as xstage,
            tc.tile_pool(name="xbuf", bufs=1) as xbufp,
            tc.tile_pool(name="wstripe", bufs=3) as wstripep,
            tc.tile_pool(name="scb", bufs=2) as scbp,
            tc.tile_pool(name="bcb", bufs=2) as bcbp,
            tc.tile_pool(name="w2", bufs=3) as w2p,
            tc.tile_pool(name="ostage", bufs=3) as ostagep,
            tc.tile_pool(name="psum", bufs=4, space="PSUM") as psump,
        ):
            mu1_sb = constp.tile([P, NT], mybir.dt.float32)
            nc.sync.dma_start(mu1_sb[:], mu1_d[:])
            bias_sb = constp.tile([P, NCH], mybir.dt.float32)
            nc.sync.dma_start(bias_sb[:], bias_d[:])

            # x prep: [N, M] fp32 -> bf16 * mu1, resident
            xbf = xbufp.tile([P, NT * M], mybir.dt.bfloat16)
            for t in range(NT):
                xs = xstage.tile([P, M], mybir.dt.float32)
                nc.sync.dma_start(xs[:], xT_d[t * P:(t + 1) * P, :])
                nc.vector.tensor_scalar_mul(
                    xbf[:, t * M:(t + 1) * M], xs[:], mu1_sb[:, t:t + 1]
                )

            for c in range(NCH):
                ws = wstripep.tile([P, L], mybir.dt.bfloat16)
                nc.sync.dma_start(ws[:], wTs_d[c])

                # host-prebroadcast per-partition scale/bias tiles for this chunk
                scb = scbp.tile([P, L], mybir.dt.bfloat16)
                nc.sync.dma_start(scb[:], zsbc_d[c, 0])
                bcb = bcbp.tile([P, L], mybir.dt.bfloat16)
                nc.sync.dma_start(bcb[:], zsbc_d[c, 1])

                # dequant: W2 = Q * s' + b'
                w2 = w2p.tile([P, L], mybir.dt.bfloat16)
                nc.vector.tensor_tensor(w2[:], ws[:], scb[:], mybir.AluOpType.mult)
                nc.vector.tensor_tensor(w2[:], w2[:], bcb[:], mybir.AluOpType.add)

                ps = psump.tile([P, M], mybir.dt.float32)
                for t in range(NT):
                    lhsT = w2[:, t * P:(t + 1) * P]
                    nc.tensor.matmul(
                        ps[:, 0:FREE],
                        lhsT,
                        xbf[:, t * M:t * M + FREE],
                        start=(t == 0),
                        stop=(t == NT - 1),
                    )
                    nc.tensor.matmul(
                        ps[:, FREE:M],
                        lhsT,
                        xbf[:, t * M + FREE:(t + 1) * M],
                        start=(t == 0),
                        stop=(t == NT - 1),
                    )

                os_ = ostagep.tile([P, M], mybir.dt.float32)
                nc.vector.tensor_scalar_add(os_[:], ps[:], bias_sb[:, c:c + 1])
                nc.sync.dma_start(outT_d[c * P:(c + 1) * P, :], os_[:])

    nc.compile()
    return nc


def _get_nc():
    global _NC_CACHE
    if _NC_CACHE is None:
        _NC_CACHE = _build_program()
    return _NC_CACHE


def _host_prep(x, scales, zeros, mu1, mu2, bias, W_q):
    x = np.asarray(x, dtype=np.float32)
    scales = np.asarray(scales, dtype=np.float32)
    zeros = np.asarray(zeros, dtype=np.float32)
    mu1 = np.asarray(mu1, dtype=np.float32)
    mu2 = np.asarray(mu2, dtype=np.float32)
    bias = np.asarray(bias, dtype=np.float32)
    W_q = np.asarray(W_q)

    # x -> transposed [N, M_TOT], sharded along tokens
    xT = np.ascontiguousarray(x.reshape(M_TOT, N).T)

    # W^T bf16 (lossless for 0..15), swizzled chunk-major:
    # wTs[c, p, t*P + j] = W_q.T[t*P + p, c*P + j]
    W8 = W_q.T.astype(BF16)                       # [N, K]
    wTs = np.ascontiguousarray(
        W8.reshape(NT, P, NCH, P).transpose(2, 1, 0, 3)
    ).reshape(NCH, P, L)

    # per-group scale/bias rows, chunk-major, split by group parity
    s2 = scales[:, :, 0] * mu2[:, None]           # [K, 64]
    b2 = -(zeros[:, :, 0] * s2)                   # [K, 64]

    def chunk_major(rowsT):                       # rowsT: [32, K]
        return rowsT.reshape(NT, NCH, P).transpose(1, 0, 2).reshape(NCH, L)

    sT = s2.T                                     # [64, K]
    bT = b2.T

    def prebroadcast(rowsT):                      # rowsT: [64, K] -> [NCH, P, L]
        lo = np.broadcast_to(chunk_major(rowsT[0::2])[:, None, :], (NCH, GS, L))
        hi = np.broadcast_to(chunk_major(rowsT[1::2])[:, None, :], (NCH, GS, L))
        return np.concatenate([lo, hi], axis=1)

    zsbc = np.ascontiguousarray(
        np.stack([prebroadcast(sT), prebroadcast(bT)], axis=1).astype(BF16)
    )                                             # [NCH, 2, P, L]

    mu1c = np.ascontiguousarray(mu1.reshape(NT, P).T)    # [P, NT]
    biasc = np.ascontiguousarray(bias.reshape(NCH, P).T)  # [P, NCH]

    in_maps = []
    for i in range(NCORES):
        in_maps.append(
            {
                "xT": np.ascontiguousarray(xT[:, i * M:(i + 1) * M]),
                "wTs": wTs,
                "zsbc": zsbc,
                "mu1c": mu1c,
                "biasc": biasc,
            }
        )
    return in_maps


def run(inputs, trace=False):
    nc = _get_nc()
    in_maps = _host_prep(**inputs)
    res = run_bass_kernel_spmd(
        nc,
        in_maps,
        list(range(NCORES)),
        trace=trace,
        trace_cores=[0] if trace else None,
    )
    outT_full = np.concatenate(
        [np.asarray(res.results[i]["outT"]) for i in range(NCORES)], axis=1
    )  # [K, M_TOT]
    out = np.ascontiguousarray(outT_full.T).reshape(4, 2048, K).astype(np.float32)
    return out, res


def kernel(**inputs):
    out, _ = run(inputs, trace=False)
    return out


# revision 9
# speedup vs baseline: 16.6438x; 16.6438x over previous
"""Trainium2 Bass kernel for group-quantized linear layer (GCLIQuantizedLinear).

Computes out[b,s,k] = sum_n x[b,s,n] * W_deq[k,n] + bias[k] where
W_deq = ((W_q - zeros) * scales) * mu2[:,None] * mu1[None,:].

Sharding: data-parallel over the 8192 tokens (M) across 8 cores; every core
holds the full weight matrix. Per core:
  - x shard arrives transposed [N=4096, M=1024] fp32, scaled by mu1 and cast
    to bf16 on device (per-partition tensor_scalar).
  - W_q arrives as W^T bf16 (values 0..15, lossless), host-swizzled so each
    128-column k-chunk is one contiguous 1 MiB DMA in the exact SBUF layout
    [128 n-part, 32 n-tiles, 128 k].
  - Dequant on DVE: W2 = Q * s' + b' with s' = scales*mu2, b' = -zeros*scales*mu2,
    materialized as [128, ...] tiles via partition-broadcast DMAs (stride-0 src).
  - TensorE: out^T[k-chunk, m] accumulated over 32 n-tiles in PSUM,
    bias added during PSUM->SBUF evacuation (per-partition tensor_scalar_add).
Host reassembles out^T columns -> [8192, 4096] -> [4,2048,4096].
"""

import sys

if "/opt/trn_rl_repo" not in sys.path:
    sys.path.insert(0, "/opt/trn_rl_repo")

import numpy as np
import ml_dtypes

import concourse.bass as bass
import concourse.tile as tile
from concourse import mybir, bacc
from concourse.bass_utils import run_bass_kernel_spmd

BF16 = ml_dtypes.bfloat16

P = 128          # partitions
N = 4096         # input features (contraction)
K = 4096         # output features
M_TOT = 8192     # tokens (4*2048)
NCORES = 8
M = M_TOT // NCORES          # 1024 tokens per core
NT = N // P                  # 32 n-tiles (contraction tiles)
NCH = K // P                 # 32 k-chunks of width 128
L = NT * P                   # 4096 free elems in a w-stripe
GS = 64                      # quant group size
FREE = 512                   # matmul moving free dim (one PSUM bank)

_NC_CACHE = None


def _build_program(reps=1):
    nc = bacc.Bacc("TRN2", target_bir_lowering=False, debug=False)

    xT_d = nc.dram_tensor("xT", [N, M], mybir.dt.float32, kind="ExternalInput")
    wTs_d = nc.dram_tensor("wTs", [NCH, P, L], mybir.dt.bfloat16, kind="ExternalInput")
    zsbc_d = nc.dram_tensor("zsbc", [NCH, 2, P, L], mybir.dt.bfloat16, kind="ExternalInput")
    mu1_d = nc.dram_tensor("mu1c", [P, NT], mybir.dt.float32, kind="ExternalInput")
    bias_d = nc.dram_tensor("biasc", [P, NCH], mybir.dt.float32, kind="ExternalInput")
    outT_d = nc.dram_tensor("outT", [K, M], mybir.dt.float32, kind="ExternalOutput")

    with tile.TileContext(nc) as tc:
        with (
            tc.tile_pool(name="const", bufs=1) as constp,
            tc.tile_pool(name="xstage", bufs=3) as xstage,
            tc.tile_pool(name="xbuf", bufs=1) as xbufp,
            tc.tile_pool(name="wstripe", bufs=3) as wstripep,
            tc.tile_pool(name="scb", bufs=2) as scbp,
            tc.tile_pool(name="bcb", bufs=2) as bcbp,
            tc.tile_pool(name="w2", bufs=3) as w2p,
            tc.tile_pool(name="ostage", bufs=3) as ostagep,
            tc.tile_pool(name="psum", bufs=4, space="PSUM") as psump,
        ):
            mu1_sb = constp.tile([P, NT], mybir.dt.float32)
            nc.sync.dma_start(mu1_sb[:], mu1_d[:])
            bias_sb = constp.tile([P, NCH], mybir.dt.float32)
            nc.sync.dma_start(bias_sb[:], bias_d[:])

            # x prep: [N, M] fp32 -> bf16 * mu1, resident
            xbf = xbufp.tile([P, NT * M], mybir.dt.bfloat16)
            for t in range(NT):
                xs = xstage.tile([P, M], mybir.dt.float32)
                nc.sync.dma_start(xs[:], xT_d[t * P:(t + 1) * P, :])
                nc.vector.tensor_scalar_mul(
                    xbf[:, t * M:(t + 1) * M], xs[:], mu1_sb[:, t:t + 1]
                )

            for _rep in range(reps):
              for c in range(NCH):
                ws = wstripep.tile([P, L], mybir.dt.bfloat16)
                nc.sync.dma_start(ws[:], wTs_d[c])

                # host-prebroadcast per-partition scale/bias tiles for this chunk
                scb = scbp.tile([P, L], mybir.dt.bfloat16)
                nc.sync.dma_start(scb[:], zsbc_d[c, 0])
                bcb = bcbp.tile([P, L], mybir.dt.bfloat16)
                nc.sync.dma_start(bcb[:], zsbc_d[c, 1])

                # dequant: W2 = Q * s' + b'
                w2 = w2p.tile([P, L], mybir.dt.bfloat16)
                nc.vector.tensor_tensor(w2[:], ws[:], scb[:], mybir.AluOpType.mult)
                nc.vector.tensor_tensor(w2[:], w2[:], bcb[:], mybir.AluOpType.add)

                ps = psump.tile([P, M], mybir.dt.float32)
                for t in range(NT):
                    lhsT = w2[:, t * P:(t + 1) * P]
                    nc.tensor.matmul(
                        ps[:, 0:FREE],
                        lhsT,
                        xbf[:, t * M:t * M + FREE],
                        start=(t == 0),
                        stop=(t == NT - 1),
                    )
                    nc.tensor.matmul(
                        ps[:, FREE:M],
                        lhsT,
                        xbf[:, t * M + FREE:(t + 1) * M],
                        start=(t == 0),
                        stop=(t == NT - 1),
                    )

                os_ = ostagep.tile([P, M], mybir.dt.float32)
                nc.vector.tensor_scalar_add(os_[:], ps[:], bias_sb[:, c:c + 1])
                nc.sync.dma_start(outT_d[c * P:(c + 1) * P, :], os_[:])

    nc.compile()
    return nc


def _get_nc():
    global _NC_CACHE
    if _NC_CACHE is None:
        _NC_CACHE = _build_program()
    return _NC_CACHE


def _host_prep(x, scales, zeros, mu1, mu2, bias, W_q):
    x = np.asarray(x, dtype=np.float32)
    scales = np.asarray(scales, dtype=np.float32)
    zeros = np.asarray(zeros, dtype=np.float32)
    mu1 = np.asarray(mu1, dtype=np.float32)
    mu2 = np.asarray(mu2, dtype=np.float32)
    bias = np.asarray(bias, dtype=np.float32)
    W_q = np.asarray(W_q)

    # x -> transposed [N, M_TOT], sharded along tokens
    xT = np.ascontiguousarray(x.reshape(M_TOT, N).T)

    # W^T bf16 (lossless for 0..15), swizzled chunk-major:
    # wTs[c, p, t*P + j] = W_q.T[t*P + p, c*P + j]
    W8 = W_q.T.astype(BF16)                       # [N, K]
    wTs = np.ascontiguousarray(
        W8.reshape(NT, P, NCH, P).transpose(2, 1, 0, 3)
    ).reshape(NCH, P, L)

    # per-group scale/bias rows, chunk-major, split by group parity
    s2 = scales[:, :, 0] * mu2[:, None]           # [K, 64]
    b2 = -(zeros[:, :, 0] * s2)                   # [K, 64]

    def chunk_major(rowsT):                       # rowsT: [32, K]
        return rowsT.reshape(NT, NCH, P).transpose(1, 0, 2).reshape(NCH, L)

    sT = s2.T                                     # [64, K]
    bT = b2.T

    def prebroadcast(rowsT):                      # rowsT: [64, K] -> [NCH, P, L]
        lo = np.broadcast_to(chunk_major(rowsT[0::2])[:, None, :], (NCH, GS, L))
        hi = np.broadcast_to(chunk_major(rowsT[1::2])[:, None, :], (NCH, GS, L))
        return np.concatenate([lo, hi], axis=1)

    zsbc = np.ascontiguousarray(
        np.stack([prebroadcast(sT), prebroadcast(bT)], axis=1).astype(BF16)
    )                                             # [NCH, 2, P, L]

    mu1c = np.ascontiguousarray(mu1.reshape(NT, P).T)    # [P, NT]
    biasc = np.ascontiguousarray(bias.reshape(NCH, P).T)  # [P, NCH]

    in_maps = []
    for i in range(NCORES):
        in_maps.append(
            {
                "xT": np.ascontiguousarray(xT[:, i * M:(i + 1) * M]),
                "wTs": wTs,
                "zsbc": zsbc,
                "mu1c": mu1c,
                "biasc": biasc,
            }
        )
    return in_maps


def run(inputs, trace=False):
    nc = _get_nc()
    in_maps = _host_prep(**inputs)
    res = run_bass_kernel_spmd(
        nc,
        in_maps,
        list(range(NCORES)),
        trace=trace,
        trace_cores=[0] if trace else None,
    )
    outT_full = np.concatenate(
        [np.asarray(res.results[i]["outT"]) for i in range(NCORES)], axis=1
    )  # [K, M_TOT]
    out = np.ascontiguousarray(outT_full.T).reshape(4, 2048, K).astype(np.float32)
    return out, res


def kernel(**inputs):
    out, _ = run(inputs, trace=False)
    return out


# revision 10
# speedup vs baseline: 81.3579x; 4.8882x over previous
"""Trainium2 Bass kernel for group-quantized linear layer (GCLIQuantizedLinear).

Computes out[b,s,k] = sum_n x[b,s,n] * W_deq[k,n] + bias[k] where
W_deq = ((W_q - zeros) * scales) * mu2[:,None] * mu1[None,:].

Sharding: data-parallel over the 8192 tokens (M) across 8 cores; every core
holds the full weight matrix. Per core:
  - x shard arrives transposed [N=4096, M=1024] fp32, scaled by mu1 and cast
    to bf16 on device (per-partition tensor_scalar).
  - W_q arrives as W^T bf16 (values 0..15, lossless), host-swizzled so each
    128-column k-chunk is one contiguous 1 MiB DMA in the exact SBUF layout
    [128 n-part, 32 n-tiles, 128 k].
  - Dequant on DVE: W2 = Q * s' + b' with s' = scales*mu2, b' = -zeros*scales*mu2,
    materialized as [128, ...] tiles via partition-broadcast DMAs (stride-0 src).
  - TensorE: out^T[k-chunk, m] accumulated over 32 n-tiles in PSUM,
    bias added during PSUM->SBUF evacuation (per-partition tensor_scalar_add).
Host reassembles out^T columns -> [8192, 4096] -> [4,2048,4096].
"""

import sys

if "/opt/trn_rl_repo" not in sys.path:
    sys.path.insert(0, "/opt/trn_rl_repo")

import numpy as np
import ml_dtypes

import concourse.bass as bass
import concourse.tile as tile
from concourse import mybir, bacc
from concourse.bass_utils import run_bass_kernel_spmd

BF16 = ml_dtypes.bfloat16

P = 128          # partitions
N = 4096         # input features (contraction)
K = 4096         # output features
M_TOT = 8192     # tokens (4*2048)
NCORES = 8
M = M_TOT // NCORES          # 1024 tokens per core
NT = N // P                  # 32 n-tiles (contraction tiles)
NCH = K // P                 # 32 k-chunks of width 128
L = NT * P                   # 4096 free elems in a w-stripe
GS = 64                      # quant group size
FREE = 512                   # matmul moving free dim (one PSUM bank)

_NC_CACHE = None


def _build_program(reps=1):
    nc = bacc.Bacc("TRN2", target_bir_lowering=False, debug=False)

    xT_d = nc.dram_tensor("xT", [N, M], mybir.dt.float32, kind="ExternalInput")
    wTs_d = nc.dram_tensor("wTs", [NCH, P, L], mybir.dt.bfloat16, kind="ExternalInput")
    zsbc_d = nc.dram_tensor("zsbc", [NCH, 2, P, L], mybir.dt.bfloat16, kind="ExternalInput")
    mu1_d = nc.dram_tensor("mu1c", [P, NT], mybir.dt.float32, kind="ExternalInput")
    bias_d = nc.dram_tensor("biasc", [P, NCH], mybir.dt.float32, kind="ExternalInput")
    outT_d = nc.dram_tensor("outT", [K, M], mybir.dt.float32, kind="ExternalOutput")

    with tile.TileContext(nc) as tc:
        with (
            tc.tile_pool(name="const", bufs=1) as constp,
            tc.tile_pool(name="xstage", bufs=3) as xstage,
            tc.tile_pool(name="xbuf", bufs=1) as xbufp,
            tc.tile_pool(name="wstripe", bufs=3) as wstripep,
            tc.tile_pool(name="scb", bufs=2) as scbp,
            tc.tile_pool(name="bcb", bufs=2) as bcbp,
            tc.tile_pool(name="w2", bufs=3) as w2p,
            tc.tile_pool(name="ostage", bufs=3) as ostagep,
            tc.tile_pool(name="psum", bufs=4, space="PSUM") as psump,
        ):
            mu1_sb = constp.tile([P, NT], mybir.dt.float32)
            nc.sync.dma_start(mu1_sb[:], mu1_d[:])
            bias_sb = constp.tile([P, NCH], mybir.dt.float32)
            nc.sync.dma_start(bias_sb[:], bias_d[:])

            # x prep: [N, M] fp32 -> bf16 * mu1, resident
            xbf = xbufp.tile([P, NT * M], mybir.dt.bfloat16)
            for t in range(NT):
                xs = xstage.tile([P, M], mybir.dt.float32)
                nc.sync.dma_start(xs[:], xT_d[t * P:(t + 1) * P, :])
                nc.vector.tensor_scalar_mul(
                    xbf[:, t * M:(t + 1) * M], xs[:], mu1_sb[:, t:t + 1]
                )

            for _rep in range(reps):
              for c in range(NCH):
                ws = wstripep.tile([P, L], mybir.dt.bfloat16)
                nc.sync.dma_start(ws[:], wTs_d[c])

                # host-prebroadcast per-partition scale/bias tiles for this chunk
                scb = scbp.tile([P, L], mybir.dt.bfloat16)
                nc.sync.dma_start(scb[:], zsbc_d[c, 0])
                bcb = bcbp.tile([P, L], mybir.dt.bfloat16)
                nc.sync.dma_start(bcb[:], zsbc_d[c, 1])

                # dequant: W2 = Q * s' + b'
                w2 = w2p.tile([P, L], mybir.dt.bfloat16)
                nc.vector.tensor_tensor(w2[:], ws[:], scb[:], mybir.AluOpType.mult)
                nc.vector.tensor_tensor(w2[:], w2[:], bcb[:], mybir.AluOpType.add)

                ps = psump.tile([P, M], mybir.dt.float32)
                for t in range(NT):
                    lhsT = w2[:, t * P:(t + 1) * P]
                    nc.tensor.matmul(
                        ps[:, 0:FREE],
                        lhsT,
                        xbf[:, t * M:t * M + FREE],
                        start=(t == 0),
                        stop=(t == NT - 1),
                    )
                    nc.tensor.matmul(
                        ps[:, FREE:M],
                        lhsT,
                        xbf[:, t * M + FREE:(t + 1) * M],
                        start=(t == 0),
                        stop=(t == NT - 1),
                    )

                os_ = ostagep.tile([P, M], mybir.dt.float32)
                nc.vector.tensor_scalar_add(os_[:], ps[:], bias_sb[:, c:c + 1])
                nc.sync.dma_start(outT_d[c * P:(c + 1) * P, :], os_[:])

    nc.compile()
    return nc


def _get_nc():
    global _NC_CACHE
    if _NC_CACHE is None:
        _NC_CACHE = _build_program()
    return _NC_CACHE


def _host_prep(x, scales, zeros, mu1, mu2, bias, W_q):
    x = np.asarray(x, dtype=np.float32)
    scales = np.asarray(scales, dtype=np.float32)
    zeros = np.asarray(zeros, dtype=np.float32)
    mu1 = np.asarray(mu1, dtype=np.float32)
    mu2 = np.asarray(mu2, dtype=np.float32)
    bias = np.asarray(bias, dtype=np.float32)
    W_q = np.asarray(W_q)

    # x -> transposed [N, M_TOT], sharded along tokens
    xT = np.ascontiguousarray(x.reshape(M_TOT, N).T)

    # W^T bf16 (lossless for 0..15), swizzled chunk-major:
    # wTs[c, p, t*P + j] = W_q.T[t*P + p, c*P + j]
    W8 = W_q.T.astype(BF16)                       # [N, K]
    wTs = np.ascontiguousarray(
        W8.reshape(NT, P, NCH, P).transpose(2, 1, 0, 3)
    ).reshape(NCH, P, L)

    # per-group scale/bias rows, chunk-major, split by group parity
    s2 = scales[:, :, 0] * mu2[:, None]           # [K, 64]
    b2 = -(zeros[:, :, 0] * s2)                   # [K, 64]

    def chunk_major(rowsT):                       # rowsT: [32, K]
        return rowsT.reshape(NT, NCH, P).transpose(1, 0, 2).reshape(NCH, L)

    sT = s2.T                                     # [64, K]
    bT = b2.T

    def prebroadcast(rowsT):                      # rowsT: [64, K] -> [NCH, P, L]
        lo = np.broadcast_to(chunk_major(rowsT[0::2])[:, None, :], (NCH, GS, L))
        hi = np.broadcast_to(chunk_major(rowsT[1::2])[:, None, :], (NCH, GS, L))
        return np.concatenate([lo, hi], axis=1)

    zsbc = np.ascontiguousarray(
        np.stack([prebroadcast(sT), prebroadcast(bT)], axis=1).astype(BF16)
    )                                             # [NCH, 2, P, L]

    mu1c = np.ascontiguousarray(mu1.reshape(NT, P).T)    # [P, NT]
    biasc = np.ascontiguousarray(bias.reshape(NCH, P).T)  # [P, NCH]

    in_maps = []
    for i in range(NCORES):
        in_maps.append(
            {
                "xT": np.ascontiguousarray(xT[:, i * M:(i + 1) * M]),
                "wTs": wTs,
                "zsbc": zsbc,
                "mu1c": mu1c,
                "biasc": biasc,
            }
        )
    return in_maps


def run(inputs, trace=False):
    nc = _get_nc()
    in_maps = _host_prep(**inputs)
    last_err = None
    for attempt in range(3):
        try:
            res = run_bass_kernel_spmd(
                nc,
                in_maps,
                list(range(NCORES)),
                trace=trace,
                trace_cores=[0] if trace else None,
            )
            break
        except Exception as e:  # transient NRT device errors — retry
            last_err = e
            import time as _time

            _time.sleep(5.0)
    else:
        raise last_err
    outT_full = np.concatenate(
        [np.asarray(res.results[i]["outT"]) for i in range(NCORES)], axis=1
    )  # [K, M_TOT]
    out = np.ascontiguousarray(outT_full.T).reshape(4, 2048, K).astype(np.float32)
    return out, res


def kernel(**inputs):
    out, _ = run(inputs, trace=False)
    return out
